# revision 17
# baseline (speedup 1.0000x reference)
"""MoE kernel for nn_MoE_1984274891212 on 8 trn2 NeuronCores.

Expert-parallel, bf16 compute (fp32 PSUM accumulation), fp32 router.

Structure per core (SPMD, one compiled program):
  - Router runs on the core's OWN 1024-token shard only, in fp32 (exact
    top-4: min 4th/5th rel score gap 4.9e-5 >> fp32-PE error ~1e-7).
    Top-4 masks for all 16 experts are written transposed [16, 1024] and
    AllGathered -> mskT_all [128, 8192].
  - Dispatch: each core reads the mask rows for its 2 experts (indirect
    row gather via host-provided row ids), transposes to [128tok, 64tile]
    columns, prefix-sums via triangular matmuls into compacted slots, and
    scatters global token ids into tid arrays (capacity = exact per-slot
    max count rounded to 128; experts ranked by count: slot0 = ranks 0-7,
    slot1 = ranks 8-15 -> identical work per core).
  - Expert job (used for 2 routed experts AND the shared expert split as
    two dense I-ranges over the own shard): x rows gathered by tid
    (routed) or xT loaded directly (shared), PE-transposed to xT tiles,
    gate/up in bf16 with fp32 PSUM, silu*u, down-proj emitted token-major
    via lhsT=h (no output transposes), scatter-added into py [8192,2048]
    bf16 by token id (routed) or written densely to zsh0 (shared; second
    half accumulates via DMA accum_op=add).
  - Shared ibs 0..13 run FIRST, with gate/up split around the dispatch
    emission so the PE never stalls on the AllGather / id-scatter chain.
    Shared ibs 13..22 run LAST, emitted before the ReduceScatter call so
    they overlap it (the CC engine runs independently).
  - out = RS(py) + zsh0 in fp32.

Assumes extra_scale == 0 and extra_bias == 0 (checked at run time; true
for this problem's inputs): combine weights are exactly 1.0 and top-4 on
raw |u*silu(g)| equals top-4 on biased softmax probs.
"""
import numpy as np

import concourse.bass as bass
import concourse.mybir as mybir
import concourse.tile as tile
import concourse.tile_utils as tile_utils
from concourse.dt import dt as cdt
from concourse.masks import make_identity
from concourse.alu_op_type import AluOpType
from concourse.bass_utils import run_bass_kernel_spmd

P = 128
T = 8192
H = 2048
E = 16
K = 4
I = 1408
ISH = 2816           # shared expert intermediate (2*I)
NT = T // P          # 64 token tiles
NCORES = 8
TSH = T // NCORES    # 1024 tokens per core shard
NTS = TSH // P       # 8 shard tiles
NHS = H // P         # 16 contraction slices
NIB = I // P         # 11 I blocks per routed expert
NS0 = 13             # shared ib count in first chunk
NS1 = (ISH // P) - NS0   # = 9, second chunk
BIG = 1 << 20

f32 = mybir.dt.float32
bf16 = mybir.dt.bfloat16
i32 = mybir.dt.int32
AF = mybir.ActivationFunctionType
np_bf16 = cdt.np(bf16)

_cached = {}

tile_utils.max_sbuf_usage = 208 * 1024

# ---------------------------------------------------------------------------
# walrus workaround: this build allows only ONE sync-wait per instruction;
# move extra waits onto standalone NoOps on the same engine.
_wctr = [0]


def _split_multi_waits(nc):
    for fn in nc.m.functions:
        for bb in fn.blocks:
            insts = bb.instructions
            out = []
            changed = False
            for inst in insts:
                si = inst.sync_info
                if si is not None and len(si.on_wait) > 1:
                    waits = list(si.on_wait)
                    for w in waits[:-1]:
                        _wctr[0] += 1
                        nop = mybir.InstNoOp(name=f"WSPLIT-{_wctr[0]}")
                        nop.engine = inst.engine
                        nop.sync_info = mybir.SyncInfo(on_wait=[w], on_update=[])
                        out.append(nop)
                    inst.sync_info = mybir.SyncInfo(
                        on_wait=[waits[-1]], on_update=list(si.on_update)
                    )
                    changed = True
                out.append(inst)
            if changed:
                bb.instructions = out
# ---------------------------------------------------------------------------


def build(CT0, CT1):
    CTs = [CT0, CT1]
    CTM = max(CT0, CT1)
    CE = CTM * P

    nc = bass.Bass()
    xbf = nc.dram_tensor("xbf", [T, H], bf16, kind="ExternalInput")
    xtf = nc.dram_tensor("xtf", [H, TSH], f32, kind="ExternalInput")
    xtb = nc.dram_tensor("xtb", [H, TSH], bf16, kind="ExternalInput")
    rwp = nc.dram_tensor("rwp", [P, NHS * 32], f32, kind="ExternalInput")
    goff = nc.dram_tensor("goff", [NT], i32, kind="ExternalInput")
    erows = nc.dram_tensor("erows", [16, 1], i32, kind="ExternalInput")
    JGU = [nc.dram_tensor(f"JGU{j}", [P, NIB * 2 * NHS * P], bf16,
                          kind="ExternalInput") for j in range(2)]
    JD = [nc.dram_tensor(f"JD{j}", [I, H], bf16, kind="ExternalInput")
          for j in range(2)]
    SGU = [nc.dram_tensor(f"SGU{j}", [P, nib * 2 * NHS * P], bf16,
                          kind="ExternalInput")
           for j, nib in ((0, NS0), (1, NS1))]
    SD = [nc.dram_tensor(f"SD{j}", [nib * P, H], bf16, kind="ExternalInput")
          for j, nib in ((0, NS0), (1, NS1))]
    out = nc.dram_tensor("out", [TSH, H], f32, kind="ExternalOutput")

    mskT_sh = nc.dram_tensor("mskT_sh", [16, TSH], f32)
    mskT_all = nc.dram_tensor("mskT_all", [NCORES * 16, TSH], f32)
    baseb = [nc.dram_tensor(f"baseb{e}", [NT], f32) for e in range(2)]
    tid_d = [nc.dram_tensor(f"tid{e}", [CTs[e] * P, 1], i32) for e in range(2)]
    py = nc.dram_tensor("py", [T, H], bf16)
    zsh0 = nc.dram_tensor("zsh0", [TSH, H], bf16)
    rs_out = nc.dram_tensor("rs_out", [TSH, H], bf16)

    with tile.TileContext(nc) as tc:
        with tc.tile_pool(name="const", bufs=1) as cpool, \
             tc.tile_pool(name="xts", bufs=1) as xtp, \
             tc.tile_pool(name="hts", bufs=1) as htp, \
             tc.tile_pool(name="wgu", bufs=2) as wp, \
             tc.tile_pool(name="wd", bufs=1) as wdp, \
             tc.tile_pool(name="dt", bufs=2) as dtp, \
             tc.tile_pool(name="yb", bufs=2) as ybp, \
             tc.tile_pool(name="tid", bufs=1) as tidp, \
             tc.tile_pool(name="sm", bufs=2) as sm, \
             tc.tile_pool(name="fin", bufs=2) as finp, \
             tc.tile_pool(name="ps", bufs=2, space="PSUM") as ps, \
             tc.tile_pool(name="psy", bufs=2, space="PSUM") as psyp, \
             tc.tile_pool(name="pst", bufs=2, space="PSUM") as pst:

            # ---------------- constants ----------------
            identF = cpool.tile([P, P], f32)
            make_identity(nc, identF[:])
            identB = cpool.tile([P, P], bf16)
            make_identity(nc, identB[:])
            # triEX[k, p] = 1 iff k < p (strict lower -> exclusive prefix)
            triEX = cpool.tile([P, P], f32)
            nc.gpsimd.memset(triEX[:], 0.0)
            nc.gpsimd.affine_select(
                out=triEX[:], in_=triEX[:], compare_op=AluOpType.is_ge,
                fill=1.0, base=0, pattern=[[-1, P]], channel_multiplier=1)
            ones_col = cpool.tile([P, 1], f32)
            nc.vector.memset(ones_col[:], 1.0)
            zt = cpool.tile([P, 1024], bf16)
            nc.vector.memset(zt[:], 0.0)
            sent = cpool.tile([P, CTM], i32)
            nc.vector.memset(sent[:], BIG)
            goff_bc = cpool.tile([P, NT], i32)
            nc.scalar.dma_start(out=goff_bc[:],
                                in_=bass.AP(goff, 0, [[0, P], [1, NT]]))
            erows_sb = [cpool.tile([8, 1], i32, name=f"erows_sb{e}")
                        for e in range(2)]
            for e in range(2):
                nc.scalar.dma_start(out=erows_sb[e][:],
                                    in_=erows[e * 8:(e + 1) * 8, :])
            idc = cpool.tile([P, NT], i32)
            nc.gpsimd.iota(idc[:], pattern=[[0, NT]], base=0,
                           channel_multiplier=1)
            nc.vector.tensor_add(out=idc[:], in0=idc[:], in1=goff_bc[:])

            breg_t = nc.gpsimd.to_reg(T - 1)
            breg_e = [nc.gpsimd.to_reg(CTs[e] * P - 1) for e in range(2)]

            # zero py; sentinel-init tid arrays (scalar queue: keep sync free)
            for i in range(NT):
                for hh in range(2):
                    nc.scalar.dma_start(
                        out=py[i * P:(i + 1) * P, hh * 1024:(hh + 1) * 1024],
                        in_=zt[:])
            for e in range(2):
                nc.scalar.dma_start(
                    out=tid_d[e][:, :].rearrange("(a p) m -> p (a m)", p=P),
                    in_=sent[:, :CTs[e]])

            # ---------------- router on own shard (fp32) ----------------
            rw_sb = sm.tile([P, NHS * 32], f32, tag="rw", name="rw_sb", bufs=1)
            nc.sync.dma_start(out=rw_sb[:], in_=rwp[:, :])
            scT = sm.tile([32, TSH], f32, tag="scr", name="scT", bufs=1)
            for ch in range(2):
                psc = psyp.tile([P, 512], f32, tag="py")
                for hs in range(NHS):
                    xtf_t = sm.tile([P, 512], f32, tag="xtf",
                                    name=f"xtf_{ch}_{hs}", bufs=2)
                    nc.sync.dma_start(
                        out=xtf_t[:],
                        in_=xtf[hs * P:(hs + 1) * P, ch * 512:(ch + 1) * 512])
                    nc.tensor.matmul(
                        out=psc[:32, :], lhsT=rw_sb[:, hs * 32:(hs + 1) * 32],
                        rhs=xtf_t[:],
                        start=(hs == 0), stop=(hs == NHS - 1))
                nc.vector.tensor_copy(out=scT[:, ch * 512:(ch + 1) * 512],
                                      in_=psc[:32, :])
            mskT_sb = sm.tile([16, TSH], f32, tag="mskT", name="mskT_sb",
                              bufs=1)
            for j in range(NTS):
                tp1 = pst.tile([P, 512], f32, tag="tp")
                nc.tensor.transpose(out=tp1[:, :32],
                                    in_=scT[:, j * P:(j + 1) * P],
                                    identity=identF[:32, :32])
                gu = sm.tile([P, 32], f32, tag="gu", name=f"gu{j}")
                nc.vector.tensor_copy(out=gu[:], in_=tp1[:, :32])
                sg = sm.tile([P, 16], f32, tag="sg16", name=f"sg16_{j}")
                nc.scalar.activation(out=sg[:], in_=gu[:, 0:16], func=AF.Sigmoid)
                sc = sm.tile([P, 16], f32, tag="sc16", name=f"sc16_{j}")
                nc.vector.tensor_mul(out=sc[:], in0=gu[:, 0:16], in1=sg[:])
                nc.vector.tensor_mul(out=sc[:], in0=sc[:], in1=gu[:, 16:32])
                nc.scalar.activation(out=sc[:], in_=sc[:], func=AF.Abs)
                mr = sm.tile([P, 8], f32, tag="mr8", name=f"mr8_{j}")
                nc.vector.max(out=mr[:], in_=sc[:])
                nc.vector.memset(mr[:, K:8], -1.0)
                rep = sm.tile([P, 16], f32, tag="rep16", name=f"rep16_{j}")
                nc.vector.match_replace(out=rep[:], in_to_replace=mr[:],
                                        in_values=sc[:], imm_value=-1.0)
                msk = sm.tile([P, 16], f32, tag="msk16", name=f"msk16_{j}")
                nc.vector.tensor_scalar(out=msk[:], in0=rep[:], scalar1=-1.0,
                                        scalar2=None, op0=AluOpType.is_equal)
                tp2 = pst.tile([P, 512], f32, tag="tp")
                nc.tensor.transpose(out=tp2[:16, :P], in_=msk[:],
                                    identity=identF[:])
                nc.vector.tensor_copy(out=mskT_sb[:, j * P:(j + 1) * P],
                                      in_=tp2[:16, :P])
            nc.sync.dma_start(out=mskT_sh[:, :], in_=mskT_sb[:])

            # ---------------- AllGather masks ----------------
            nc.gpsimd.collective_compute(
                "AllGather", AluOpType.bypass,
                replica_groups=[list(range(NCORES))],
                ins=[bass.AP(mskT_sh, 0, [[TSH, 16], [1, TSH]])],
                outs=[bass.AP(mskT_all, 0, [[TSH, NCORES * 16], [1, TSH]])],
            )

            # ---------------- expert job phases ----------------
            def job_begin(jn, n_tiles, routed, e_slot, nib):
                st = {"jn": jn, "n": n_tiles, "routed": routed,
                      "e": e_slot, "nib": nib}
                W = n_tiles * P
                chunks = []
                c0 = 0
                while c0 < W:
                    w = min(512, W - c0)
                    chunks.append((c0, w))
                    c0 += w
                st["chunks"] = chunks
                xts = xtp.tile([P, NHS, CE], bf16, tag="xts", name=f"xts_{jn}")
                st["xts"] = xts
                st["tid"] = []
                st["dt"] = []
                if routed:
                    # fetch only: tid loads + x-row gathers on the gpsimd
                    # queue (right behind this expert's id-scatters; keeps
                    # the sync queue free for weight streams)
                    for g in range(n_tiles):
                        tidt = tidp.tile([P, 1], i32, tag=f"t{e_slot}_{g}",
                                         name=f"tid_{jn}_{g}")
                        nc.gpsimd.dma_start(
                            out=tidt[:],
                            in_=tid_d[e_slot][g * P:(g + 1) * P, :])
                        st["tid"].append(tidt)
                        dt_ = dtp.tile([P, H], bf16, tag="dt",
                                       name=f"dt_{jn}_{g}")
                        nc.gpsimd.indirect_dma_start(
                            out=dt_[:, :], out_offset=None,
                            in_=xbf[:, :],
                            in_offset=bass.IndirectOffsetOnAxis(
                                ap=tidt[:, 0:1], axis=0),
                            bounds_check=breg_t, oob_is_err=False)
                        st["dt"].append(dt_)
                else:
                    for hs in range(NHS):
                        nc.sync.dma_start(out=xts[:, hs, :TSH],
                                          in_=xtb[hs * P:(hs + 1) * P, :])
                st["hts"] = htp.tile([P, nib, CE], bf16, tag="hts",
                                     name=f"hts_{jn}")
                return st

            def job_transpose(st):
                xts = st["xts"]
                for g, dt_ in enumerate(st["dt"]):
                    for hq in range(2):
                        tp_ = psyp.tile([P, 1024], bf16, tag="py")
                        for k in range(8):
                            hs = hq * 8 + k
                            nc.tensor.transpose(
                                out=tp_[:, k * P:(k + 1) * P],
                                in_=dt_[:, hs * P:(hs + 1) * P],
                                identity=identB[:])
                        nc.vector.tensor_copy(
                            out=xts[:, hq * 8:(hq + 1) * 8,
                                    g * P:(g + 1) * P],
                            in_=tp_[:].rearrange("p (a m) -> p a m", a=8))

            def job_gateup(st, gu_dram, ib0, ib1):
                jn, xts, hts = st["jn"], st["xts"], st["hts"]
                for ib in range(ib0, ib1):
                    wgu = wp.tile([P, 2 * NHS * P], bf16, tag="wgu",
                                  name=f"wgu_{jn}_{ib}")
                    nc.sync.dma_start(
                        out=wgu[:],
                        in_=gu_dram[:, ib * (2 * NHS * P):
                                    (ib + 1) * (2 * NHS * P)])
                    for (c0, w) in st["chunks"]:
                        pg = ps.tile([P, 512], f32, tag="pg")
                        pu = ps.tile([P, 512], f32, tag="pu")
                        for hs in range(NHS):
                            nc.tensor.matmul(
                                out=pg[:, :w], lhsT=wgu[:, hs * P:(hs + 1) * P],
                                rhs=xts[:, hs, c0:c0 + w],
                                start=(hs == 0), stop=(hs == NHS - 1))
                        for hs in range(NHS):
                            nc.tensor.matmul(
                                out=pu[:, :w],
                                lhsT=wgu[:, (NHS + hs) * P:(NHS + hs + 1) * P],
                                rhs=xts[:, hs, c0:c0 + w],
                                start=(hs == 0), stop=(hs == NHS - 1))
                        sg_ = sm.tile([P, 512], bf16, tag="sgh",
                                      name=f"sg_{jn}_{ib}_{c0}")
                        nc.scalar.activation(out=sg_[:, :w], in_=pg[:, :w],
                                             func=AF.Silu)
                        nc.vector.tensor_mul(out=hts[:, ib, c0:c0 + w],
                                             in0=sg_[:, :w], in1=pu[:, :w])

            def job_down(st, d_dram, accum):
                jn, hts, nib = st["jn"], st["hts"], st["nib"]
                for pass_ in range(2):
                    wds = []
                    for ib in range(nib):
                        wd = wdp.tile([P, 1024], bf16, tag=f"wd{ib}",
                                      name=f"wd_{jn}_{pass_}_{ib}")
                        nc.sync.dma_start(
                            out=wd[:],
                            in_=d_dram[ib * P:(ib + 1) * P,
                                       pass_ * 1024:(pass_ + 1) * 1024])
                        wds.append(wd)
                    for t in range(st["n"]):
                        yb = ybp.tile([P, 1024], bf16, tag="yb",
                                      name=f"yb_{jn}_{pass_}_{t}")
                        for hcl in range(2):
                            psy = psyp.tile([P, 512], f32, tag="py")
                            for ib in range(nib):
                                nc.tensor.matmul(
                                    out=psy[:],
                                    lhsT=hts[:, ib, t * P:(t + 1) * P],
                                    rhs=wds[ib][:, hcl * 512:(hcl + 1) * 512],
                                    start=(ib == 0), stop=(ib == nib - 1))
                            nc.scalar.copy(out=yb[:, hcl * 512:(hcl + 1) * 512],
                                           in_=psy[:])
                        if st["routed"]:
                            nc.gpsimd.indirect_dma_start(
                                out=py[:, :],
                                out_offset=bass.IndirectOffsetOnAxis(
                                    ap=st["tid"][t][:, 0:1], axis=0),
                                in_=yb[:, :], in_offset=None,
                                element_offset=pass_ * 1024,
                                bounds_check=breg_t, oob_is_err=False,
                                compute_op=AluOpType.add)
                        elif accum:
                            nc.gpsimd.dma_start(
                                out=zsh0[t * P:(t + 1) * P,
                                         pass_ * 1024:(pass_ + 1) * 1024],
                                in_=yb[:], accum_op=AluOpType.add)
                        else:
                            nc.sync.dma_start(
                                out=zsh0[t * P:(t + 1) * P,
                                         pass_ * 1024:(pass_ + 1) * 1024],
                                in_=yb[:])

            # shared chunk 0: start gate/up before the dispatch emission so
            # the PE has work while the AllGather + id-scatters run
            s0 = job_begin("s0", NTS, False, 0, NS0)
            job_gateup(s0, SGU[0], 0, 4)

            # ---------------- dispatch: slots + id scatter ----------------
            mrows = [sm.tile([8, TSH], f32, tag="scr", name=f"mrows{e}",
                             bufs=1) for e in range(2)]
            for e in range(2):
                nc.gpsimd.indirect_dma_start(
                    out=mrows[e][:, :],
                    out_offset=None,
                    in_=mskT_all[:, :],
                    in_offset=bass.IndirectOffsetOnAxis(
                        ap=erows_sb[e][:, 0:1], axis=0),
                )

            def dispatch_expert(e):
                mask_cols = sm.tile([P, NT], f32, tag=f"mc{e}", name=f"mc{e}",
                                    bufs=1)
                for j in range(NTS):
                    tp3 = pst.tile([P, 512], f32, tag="tp")
                    nc.tensor.transpose(
                        out=tp3[:, :8],
                        in_=mrows[e][:, j * P:(j + 1) * P],
                        identity=identF[:8, :8])
                    nc.vector.tensor_copy(out=mask_cols[:, j * 8:(j + 1) * 8],
                                          in_=tp3[:, :8])
                excl_ps = pst.tile([P, 512], f32, tag="tp")
                nc.tensor.matmul(out=excl_ps[:, :NT], lhsT=triEX[:],
                                 rhs=mask_cols[:], start=True, stop=True)
                excl = sm.tile([P, NT], f32, tag=f"excl{e}", name=f"excl{e}",
                               bufs=1)
                nc.vector.tensor_copy(out=excl[:], in_=excl_ps[:, :NT])
                cnt_ps = pst.tile([P, 512], f32, tag="tp")
                nc.tensor.matmul(out=cnt_ps[:NT, :1], lhsT=mask_cols[:],
                                 rhs=ones_col[:], start=True, stop=True)
                cnt = sm.tile([NT, 1], f32, tag="cnt", name=f"cnt{e}")
                nc.vector.tensor_copy(out=cnt[:], in_=cnt_ps[:NT, :1])
                base_ps = pst.tile([P, 512], f32, tag="tp")
                nc.tensor.matmul(out=base_ps[:NT, :1], lhsT=triEX[:NT, :NT],
                                 rhs=cnt[:], start=True, stop=True)
                base_sb = sm.tile([NT, 1], f32, tag="cnt", name=f"base{e}")
                nc.vector.tensor_copy(out=base_sb[:], in_=base_ps[:NT, :1])
                nc.scalar.dma_start(out=baseb[e][:], in_=base_sb[:])
                base_bc = sm.tile([P, NT], f32, tag=f"bc{e}", name=f"bc{e}",
                                  bufs=1)
                nc.scalar.dma_start(out=base_bc[:],
                                    in_=bass.AP(baseb[e], 0, [[0, P], [1, NT]]))
                nc.vector.tensor_add(out=excl[:], in0=excl[:], in1=base_bc[:])
                nc.vector.tensor_scalar(out=excl[:], in0=excl[:],
                                        scalar1=float(-BIG), scalar2=None,
                                        op0=AluOpType.add)
                nc.vector.tensor_mul(out=excl[:], in0=excl[:], in1=mask_cols[:])
                nc.vector.tensor_scalar(out=excl[:], in0=excl[:],
                                        scalar1=float(BIG), scalar2=None,
                                        op0=AluOpType.add)
                si_ = sm.tile([P, NT], i32, tag=f"si{e}", name=f"si{e}", bufs=1)
                nc.vector.tensor_copy(out=si_[:], in_=excl[:])
                for col in range(NT):
                    nc.gpsimd.indirect_dma_start(
                        out=tid_d[e][:, :],
                        out_offset=bass.IndirectOffsetOnAxis(
                            ap=si_[:, col:col + 1], axis=0),
                        in_=idc[:, col:col + 1], in_offset=None,
                        bounds_check=breg_e[e], oob_is_err=False)

            # expert-0 dispatch, then its fetch goes straight onto the
            # gpsimd queue ahead of expert-1's scatters
            dispatch_expert(0)
            e0 = job_begin("e0", CT0, True, 0, NIB)
            dispatch_expert(1)

            # rest of shared chunk 0
            job_gateup(s0, SGU[0], 4, NS0)
            job_down(s0, SD[0], accum=False)

            # routed experts
            job_transpose(e0)
            job_gateup(e0, JGU[0], 0, NIB)
            job_down(e0, JD[0], accum=False)
            e1 = job_begin("e1", CT1, True, 1, NIB)
            job_transpose(e1)
            job_gateup(e1, JGU[1], 0, NIB)
            job_down(e1, JD[1], accum=False)

            # shared chunk 1 (overlaps the ReduceScatter below; emitted
            # first so its engine streams are ahead of the collective)
            s1 = job_begin("s1", NTS, False, 1, NS1)
            job_gateup(s1, SGU[1], 0, NS1)
            job_down(s1, SD[1], accum=True)

            # ---------------- ReduceScatter ----------------
            nc.gpsimd.collective_compute(
                "ReduceScatter", AluOpType.add,
                replica_groups=[list(range(NCORES))],
                ins=[bass.AP(py, 0, [[H, T], [1, H]])],
                outs=[bass.AP(rs_out, 0, [[H, TSH], [1, H]])],
            )

            # ---------------- final combine ----------------
            for g in range(NTS):
                rt = dtp.tile([P, H], bf16, tag="dt", name=f"rt{g}")
                nc.scalar.dma_start(out=rt[:], in_=rs_out[g * P:(g + 1) * P, :])
                z0 = sm.tile([P, H], bf16, tag="mskT", name=f"z0_{g}",
                             bufs=1)
                nc.scalar.dma_start(out=z0[:], in_=zsh0[g * P:(g + 1) * P, :])
                for hh in range(4):
                    ob = finp.tile([P, 512], f32, tag="ob", name=f"ob{g}_{hh}")
                    nc.vector.tensor_add(
                        out=ob[:], in0=rt[:, hh * 512:(hh + 1) * 512],
                        in1=z0[:, hh * 512:(hh + 1) * 512])
                    nc.scalar.dma_start(
                        out=out[g * P:(g + 1) * P, hh * 512:(hh + 1) * 512],
                        in_=ob[:])

    _split_multi_waits(nc)
    return nc


def _pack_gu(G, U):
    """[I', H] gate and up weights -> [P, nib*2*NHS*P] bf16 lhsT-tiled:
    per ib tile, first NHS*P cols = gate lhsT slices (hs-major), then up.
    lhsT slice hs of tile ib: [p, c] = W[ib*128+c, hs*128+p]."""
    nib = G.shape[0] // P
    out = np.empty((P, nib, 2, NHS, P), dtype=np_bf16)
    for Wm, j in ((G, 0), (U, 1)):
        W4 = Wm.reshape(nib, P, NHS, P)           # [ib, c, hs, p]
        out[:, :, j, :, :] = W4.transpose(3, 0, 2, 1)
    return np.ascontiguousarray(out.reshape(P, nib * 2 * NHS * P))


def kernel(x, rg_w, ru_w, extra_scale, extra_bias, Wg, Wu, Wd, Sg, Su, Sd):
    x = np.ascontiguousarray(np.asarray(x, dtype=np.float32))
    assert np.all(np.asarray(extra_scale) == 0.0), "kernel assumes extra_scale==0"
    assert np.all(np.asarray(extra_bias) == 0.0), "kernel assumes extra_bias==0"
    B, S, _ = x.shape
    xf = x.reshape(T, H)

    rg_w = np.asarray(rg_w, np.float32)
    ru_w = np.asarray(ru_w, np.float32)
    Wg = np.asarray(Wg, np.float32)
    Wu = np.asarray(Wu, np.float32)
    Wd = np.asarray(Wd, np.float32)
    Sg = np.asarray(Sg, np.float32)
    Su = np.asarray(Su, np.float32)
    Sd = np.asarray(Sd, np.float32)

    # host routing (fp64): exact top-4 counts -> static capacities.
    g = xf.astype(np.float64) @ rg_w.astype(np.float64).T
    u = xf.astype(np.float64) @ ru_w.astype(np.float64).T
    scores = np.abs(u * (g / (1.0 + np.exp(-g))))
    part = np.argpartition(-scores, K, axis=1)[:, :K]
    cnt = np.bincount(part.ravel(), minlength=E)
    ss = np.sort(scores, axis=1)[:, ::-1]
    gap = (ss[:, K - 1] - ss[:, K]) / np.maximum(ss[:, K - 1], 1e-30)
    assert gap.min() > 1e-5, f"top-4 margin too small: {gap.min()}"

    order = np.argsort(-cnt, kind="stable")
    CT0 = -(-int(cnt[order[0]]) // P)
    CT1 = -(-int(cnt[order[8]]) // P)

    key = (CT0, CT1)
    if _cached.get("key") != key:
        _cached.clear()
        _cached["key"] = key
        _cached["nc"] = build(CT0, CT1)
    nc = _cached["nc"]

    x_bf = xf.astype(np_bf16)
    rw32 = np.concatenate([rg_w, ru_w], axis=0)          # [32, H]
    rwp = np.ascontiguousarray(
        rw32.T.reshape(NHS, P, 32).transpose(1, 0, 2).reshape(P, NHS * 32)
    ).astype(np.float32)
    goff = (((np.arange(NT) % NTS) * TSH)
            + ((np.arange(NT) // NTS) * P)).astype(np.int32)

    SdT = np.ascontiguousarray(Sd.T)                     # [ISH, H]
    B0 = NS0 * P
    sgu_pack = [_pack_gu(Sg[:B0], Su[:B0]), _pack_gu(Sg[B0:], Su[B0:])]
    sd_pack = [np.ascontiguousarray(SdT[:B0]).astype(np_bf16),
               np.ascontiguousarray(SdT[B0:]).astype(np_bf16)]

    in_maps = []
    for c in range(NCORES):
        ea, eb = int(order[c]), int(order[15 - c])
        xsh = xf[c * TSH:(c + 1) * TSH]
        m = {
            "xbf": x_bf,
            "xtf": np.ascontiguousarray(xsh.T),
            "xtb": np.ascontiguousarray(xsh.T).astype(np_bf16),
            "rwp": rwp,
            "goff": goff,
            "erows": (np.array([cc * 16 + ea for cc in range(NCORES)]
                               + [cc * 16 + eb for cc in range(NCORES)],
                               dtype=np.int32).reshape(16, 1)),
            "JGU0": _pack_gu(Wg[ea], Wu[ea]),
            "JGU1": _pack_gu(Wg[eb], Wu[eb]),
            "JD0": np.ascontiguousarray(Wd[ea].T).astype(np_bf16),
            "JD1": np.ascontiguousarray(Wd[eb].T).astype(np_bf16),
            "SGU0": sgu_pack[0],
            "SGU1": sgu_pack[1],
            "SD0": sd_pack[0],
            "SD1": sd_pack[1],
        }
        in_maps.append(m)

    _cached["in_maps"] = in_maps
    res = run_bass_kernel_spmd(nc, in_maps, list(range(NCORES))).results
    yf = np.concatenate([res[c]["out"] for c in range(NCORES)], axis=0)
    return yf.reshape(B, S, H)


# revision 19
# speedup vs baseline: 1.0454x; 1.0454x over previous
"""MoE kernel for nn_MoE_1984274891212 on 8 trn2 NeuronCores.

Expert-parallel, bf16 compute (fp32 PSUM accumulation), fp32 router.

Structure per core (SPMD, one compiled program):
  - Router runs on the core's OWN 1024-token shard only, in fp32 (exact
    top-4: min 4th/5th rel score gap 4.9e-5 >> fp32-PE error ~1e-7).
    Top-4 masks for all 16 experts are written transposed [16, 1024] and
    AllGathered -> mskT_all [128, 8192].
  - Dispatch: each core reads the mask rows for its 2 experts (indirect
    row gather via host-provided row ids), transposes to [128tok, 64tile]
    columns, prefix-sums via triangular matmuls into compacted slots, and
    scatters global token ids into tid arrays (capacity = exact per-slot
    max count rounded to 128; experts ranked by count: slot0 = ranks 0-7,
    slot1 = ranks 8-15 -> identical work per core).
  - Expert job (used for 2 routed experts AND the shared expert split as
    two dense I-ranges over the own shard): x rows gathered by tid
    (routed) or xT loaded directly (shared), PE-transposed to xT tiles,
    gate/up in bf16 with fp32 PSUM, silu*u, down-proj emitted token-major
    via lhsT=h (no output transposes), scatter-added into py [8192,2048]
    bf16 by token id (routed) or written densely to zsh0 (shared; second
    half accumulates via DMA accum_op=add).
  - Shared ibs 0..13 run FIRST, with gate/up split around the dispatch
    emission so the PE never stalls on the AllGather / id-scatter chain.
    Shared ibs 13..22 run LAST, emitted before the ReduceScatter call so
    they overlap it (the CC engine runs independently).
  - out = RS(py) + zsh0 in fp32.

Assumes extra_scale == 0 and extra_bias == 0 (checked at run time; true
for this problem's inputs): combine weights are exactly 1.0 and top-4 on
raw |u*silu(g)| equals top-4 on biased softmax probs.
"""
import numpy as np

import concourse.bass as bass
import concourse.mybir as mybir
import concourse.tile as tile
import concourse.tile_utils as tile_utils
from concourse.dt import dt as cdt
from concourse.masks import make_identity
from concourse.alu_op_type import AluOpType
from concourse.bass_utils import run_bass_kernel_spmd

P = 128
T = 8192
H = 2048
E = 16
K = 4
I = 1408
ISH = 2816           # shared expert intermediate (2*I)
NT = T // P          # 64 token tiles
NCORES = 8
TSH = T // NCORES    # 1024 tokens per core shard
NTS = TSH // P       # 8 shard tiles
NHS = H // P         # 16 contraction slices
NIB = I // P         # 11 I blocks per routed expert
NS0 = 13             # shared ib count in first chunk
NS1 = (ISH // P) - NS0   # = 9, second chunk
BIG = 1 << 20

f32 = mybir.dt.float32
bf16 = mybir.dt.bfloat16
i32 = mybir.dt.int32
AF = mybir.ActivationFunctionType
np_bf16 = cdt.np(bf16)

_cached = {}

tile_utils.max_sbuf_usage = 208 * 1024

# ---------------------------------------------------------------------------
# walrus workaround: this build allows only ONE sync-wait per instruction;
# move extra waits onto standalone NoOps on the same engine.
_wctr = [0]


def _split_multi_waits(nc):
    for fn in nc.m.functions:
        for bb in fn.blocks:
            insts = bb.instructions
            out = []
            changed = False
            for inst in insts:
                si = inst.sync_info
                if si is not None and len(si.on_wait) > 1:
                    waits = list(si.on_wait)
                    for w in waits[:-1]:
                        _wctr[0] += 1
                        nop = mybir.InstNoOp(name=f"WSPLIT-{_wctr[0]}")
                        nop.engine = inst.engine
                        nop.sync_info = mybir.SyncInfo(on_wait=[w], on_update=[])
                        out.append(nop)
                    inst.sync_info = mybir.SyncInfo(
                        on_wait=[waits[-1]], on_update=list(si.on_update)
                    )
                    changed = True
                out.append(inst)
            if changed:
                bb.instructions = out
# ---------------------------------------------------------------------------


def build(CT0, CT1):
    CTs = [CT0, CT1]
    CTM = max(CT0, CT1)
    CE = CTM * P

    nc = bass.Bass()
    xbf = nc.dram_tensor("xbf", [T, H], bf16, kind="ExternalInput")
    xtf = nc.dram_tensor("xtf", [H, TSH], f32, kind="ExternalInput")
    xtb = nc.dram_tensor("xtb", [H, TSH], bf16, kind="ExternalInput")
    rwp = nc.dram_tensor("rwp", [P, NHS * 32], f32, kind="ExternalInput")
    goff = nc.dram_tensor("goff", [NT], i32, kind="ExternalInput")
    erows = nc.dram_tensor("erows", [16, 1], i32, kind="ExternalInput")
    JGU = [nc.dram_tensor(f"JGU{j}", [P, NIB * 2 * NHS * P], bf16,
                          kind="ExternalInput") for j in range(2)]
    JD = [nc.dram_tensor(f"JD{j}", [I, H], bf16, kind="ExternalInput")
          for j in range(2)]
    SGU = [nc.dram_tensor(f"SGU{j}", [P, nib * 2 * NHS * P], bf16,
                          kind="ExternalInput")
           for j, nib in ((0, NS0), (1, NS1))]
    SD = [nc.dram_tensor(f"SD{j}", [nib * P, H], bf16, kind="ExternalInput")
          for j, nib in ((0, NS0), (1, NS1))]
    out = nc.dram_tensor("out", [TSH, H], f32, kind="ExternalOutput")

    mskT_sh = nc.dram_tensor("mskT_sh", [16, TSH], f32)
    mskT_all = nc.dram_tensor("mskT_all", [NCORES * 16, TSH], f32)
    baseb = [nc.dram_tensor(f"baseb{e}", [NT], f32) for e in range(2)]
    tid_d = [nc.dram_tensor(f"tid{e}", [CTs[e] * P, 1], i32) for e in range(2)]
    py = nc.dram_tensor("py", [T, H], bf16)
    zsh0 = nc.dram_tensor("zsh0", [TSH, H], bf16)
    rs_out = nc.dram_tensor("rs_out", [TSH, H], bf16)

    with tile.TileContext(nc) as tc:
        with tc.tile_pool(name="const", bufs=1) as cpool, \
             tc.tile_pool(name="xts", bufs=1) as xtp, \
             tc.tile_pool(name="hts", bufs=1) as htp, \
             tc.tile_pool(name="wgu", bufs=2) as wp, \
             tc.tile_pool(name="wd", bufs=1) as wdp, \
             tc.tile_pool(name="dt", bufs=2) as dtp, \
             tc.tile_pool(name="yb", bufs=2) as ybp, \
             tc.tile_pool(name="tid", bufs=1) as tidp, \
             tc.tile_pool(name="sm", bufs=2) as sm, \
             tc.tile_pool(name="fin", bufs=2) as finp, \
             tc.tile_pool(name="ps", bufs=2, space="PSUM") as ps, \
             tc.tile_pool(name="psy", bufs=2, space="PSUM") as psyp, \
             tc.tile_pool(name="pst", bufs=2, space="PSUM") as pst:

            # ---------------- constants ----------------
            identF = cpool.tile([P, P], f32)
            make_identity(nc, identF[:])
            identB = cpool.tile([P, P], bf16)
            make_identity(nc, identB[:])
            # triEX[k, p] = 1 iff k < p (strict lower -> exclusive prefix)
            triEX = cpool.tile([P, P], f32)
            nc.gpsimd.memset(triEX[:], 0.0)
            nc.gpsimd.affine_select(
                out=triEX[:], in_=triEX[:], compare_op=AluOpType.is_ge,
                fill=1.0, base=0, pattern=[[-1, P]], channel_multiplier=1)
            ones_col = cpool.tile([P, 1], f32)
            nc.vector.memset(ones_col[:], 1.0)
            zt = cpool.tile([P, 1024], bf16)
            nc.vector.memset(zt[:], 0.0)
            sent = cpool.tile([P, CTM], i32)
            nc.vector.memset(sent[:], BIG)
            goff_bc = cpool.tile([P, NT], i32)
            nc.scalar.dma_start(out=goff_bc[:],
                                in_=bass.AP(goff, 0, [[0, P], [1, NT]]))
            erows_sb = [cpool.tile([8, 1], i32, name=f"erows_sb{e}")
                        for e in range(2)]
            for e in range(2):
                nc.scalar.dma_start(out=erows_sb[e][:],
                                    in_=erows[e * 8:(e + 1) * 8, :])
            idc = cpool.tile([P, NT], i32)
            nc.gpsimd.iota(idc[:], pattern=[[0, NT]], base=0,
                           channel_multiplier=1)
            nc.vector.tensor_add(out=idc[:], in0=idc[:], in1=goff_bc[:])

            breg_t = nc.gpsimd.to_reg(T - 1)
            breg_e = [nc.gpsimd.to_reg(CTs[e] * P - 1) for e in range(2)]

            # zero py; sentinel-init tid arrays (scalar queue: keep sync free)
            for i in range(NT):
                for hh in range(2):
                    nc.scalar.dma_start(
                        out=py[i * P:(i + 1) * P, hh * 1024:(hh + 1) * 1024],
                        in_=zt[:])
            for e in range(2):
                nc.scalar.dma_start(
                    out=tid_d[e][:, :].rearrange("(a p) m -> p (a m)", p=P),
                    in_=sent[:, :CTs[e]])

            # ---------------- router on own shard (fp32) ----------------
            rw_sb = sm.tile([P, NHS * 32], f32, tag="rw", name="rw_sb", bufs=1)
            nc.sync.dma_start(out=rw_sb[:], in_=rwp[:, :])
            xtf_sb = xtp.tile([P, NHS, TSH], f32, tag="xts", name="xtf_sb")
            for hs in range(NHS):
                nc.sync.dma_start(out=xtf_sb[:, hs, :],
                                  in_=xtf[hs * P:(hs + 1) * P, :])
            scT = sm.tile([32, TSH], f32, tag="scr", name="scT", bufs=1)
            for ch in range(2):
                psc = psyp.tile([P, 512], f32, tag="py")
                for hs in range(NHS):
                    nc.tensor.matmul(
                        out=psc[:32, :], lhsT=rw_sb[:, hs * 32:(hs + 1) * 32],
                        rhs=xtf_sb[:, hs, ch * 512:(ch + 1) * 512],
                        start=(hs == 0), stop=(hs == NHS - 1))
                nc.vector.tensor_copy(out=scT[:, ch * 512:(ch + 1) * 512],
                                      in_=psc[:32, :])
            mskT_sb = sm.tile([16, TSH], f32, tag="mskT", name="mskT_sb",
                              bufs=1)
            for j in range(NTS):
                tp1 = pst.tile([P, 512], f32, tag="tp")
                nc.tensor.transpose(out=tp1[:, :32],
                                    in_=scT[:, j * P:(j + 1) * P],
                                    identity=identF[:32, :32])
                gu = sm.tile([P, 32], f32, tag="gu", name=f"gu{j}")
                nc.vector.tensor_copy(out=gu[:], in_=tp1[:, :32])
                sg = sm.tile([P, 16], f32, tag="sg16", name=f"sg16_{j}")
                nc.scalar.activation(out=sg[:], in_=gu[:, 0:16], func=AF.Sigmoid)
                sc = sm.tile([P, 16], f32, tag="sc16", name=f"sc16_{j}")
                nc.vector.tensor_mul(out=sc[:], in0=gu[:, 0:16], in1=sg[:])
                nc.vector.tensor_mul(out=sc[:], in0=sc[:], in1=gu[:, 16:32])
                nc.scalar.activation(out=sc[:], in_=sc[:], func=AF.Abs)
                mr = sm.tile([P, 8], f32, tag="mr8", name=f"mr8_{j}")
                nc.vector.max(out=mr[:], in_=sc[:])
                nc.vector.memset(mr[:, K:8], -1.0)
                rep = sm.tile([P, 16], f32, tag="rep16", name=f"rep16_{j}")
                nc.vector.match_replace(out=rep[:], in_to_replace=mr[:],
                                        in_values=sc[:], imm_value=-1.0)
                msk = sm.tile([P, 16], f32, tag="msk16", name=f"msk16_{j}")
                nc.vector.tensor_scalar(out=msk[:], in0=rep[:], scalar1=-1.0,
                                        scalar2=None, op0=AluOpType.is_equal)
                tp2 = pst.tile([P, 512], f32, tag="tp")
                nc.tensor.transpose(out=tp2[:16, :P], in_=msk[:],
                                    identity=identF[:])
                nc.vector.tensor_copy(out=mskT_sb[:, j * P:(j + 1) * P],
                                      in_=tp2[:16, :P])
            nc.sync.dma_start(out=mskT_sh[:, :], in_=mskT_sb[:])

            # ---------------- AllGather masks ----------------
            nc.gpsimd.collective_compute(
                "AllGather", AluOpType.bypass,
                replica_groups=[list(range(NCORES))],
                ins=[bass.AP(mskT_sh, 0, [[TSH, 16], [1, TSH]])],
                outs=[bass.AP(mskT_all, 0, [[TSH, NCORES * 16], [1, TSH]])],
            )

            # ---------------- expert job phases ----------------
            def job_begin(jn, n_tiles, routed, e_slot, nib):
                st = {"jn": jn, "n": n_tiles, "routed": routed,
                      "e": e_slot, "nib": nib}
                W = n_tiles * P
                chunks = []
                c0 = 0
                while c0 < W:
                    w = min(512, W - c0)
                    chunks.append((c0, w))
                    c0 += w
                st["chunks"] = chunks
                xts = xtp.tile([P, NHS, CE], bf16, tag="xts", name=f"xts_{jn}")
                st["xts"] = xts
                st["tid"] = []
                if routed:
                    for g in range(n_tiles):
                        tidt = tidp.tile([P, 1], i32, tag=f"t{e_slot}_{g}",
                                         name=f"tid_{jn}_{g}")
                        nc.sync.dma_start(
                            out=tidt[:],
                            in_=tid_d[e_slot][g * P:(g + 1) * P, :])
                        st["tid"].append(tidt)
                        dt_ = dtp.tile([P, H], bf16, tag="dt",
                                       name=f"dt_{jn}_{g}")
                        nc.gpsimd.indirect_dma_start(
                            out=dt_[:, :], out_offset=None,
                            in_=xbf[:, :],
                            in_offset=bass.IndirectOffsetOnAxis(
                                ap=tidt[:, 0:1], axis=0),
                            bounds_check=breg_t, oob_is_err=False)
                        for hq in range(2):
                            tp_ = psyp.tile([P, 1024], bf16, tag="py")
                            for k in range(8):
                                hs = hq * 8 + k
                                nc.tensor.transpose(
                                    out=tp_[:, k * P:(k + 1) * P],
                                    in_=dt_[:, hs * P:(hs + 1) * P],
                                    identity=identB[:])
                            nc.vector.tensor_copy(
                                out=xts[:, hq * 8:(hq + 1) * 8,
                                        g * P:(g + 1) * P],
                                in_=tp_[:].rearrange("p (a m) -> p a m", a=8))
                else:
                    for hs in range(NHS):
                        nc.sync.dma_start(out=xts[:, hs, :TSH],
                                          in_=xtb[hs * P:(hs + 1) * P, :])
                st["hts"] = htp.tile([P, nib, CE], bf16, tag="hts",
                                     name=f"hts_{jn}")
                return st

            def job_gateup(st, gu_dram, ib0, ib1):
                jn, xts, hts = st["jn"], st["xts"], st["hts"]
                for ib in range(ib0, ib1):
                    wgu = wp.tile([P, 2 * NHS * P], bf16, tag="wgu",
                                  name=f"wgu_{jn}_{ib}")
                    nc.sync.dma_start(
                        out=wgu[:],
                        in_=gu_dram[:, ib * (2 * NHS * P):
                                    (ib + 1) * (2 * NHS * P)])
                    for (c0, w) in st["chunks"]:
                        pg = ps.tile([P, 512], f32, tag="pg")
                        pu = ps.tile([P, 512], f32, tag="pu")
                        for hs in range(NHS):
                            nc.tensor.matmul(
                                out=pg[:, :w], lhsT=wgu[:, hs * P:(hs + 1) * P],
                                rhs=xts[:, hs, c0:c0 + w],
                                start=(hs == 0), stop=(hs == NHS - 1))
                        for hs in range(NHS):
                            nc.tensor.matmul(
                                out=pu[:, :w],
                                lhsT=wgu[:, (NHS + hs) * P:(NHS + hs + 1) * P],
                                rhs=xts[:, hs, c0:c0 + w],
                                start=(hs == 0), stop=(hs == NHS - 1))
                        sg_ = sm.tile([P, 512], bf16, tag="sgh",
                                      name=f"sg_{jn}_{ib}_{c0}")
                        nc.scalar.activation(out=sg_[:, :w], in_=pg[:, :w],
                                             func=AF.Silu)
                        nc.vector.tensor_mul(out=hts[:, ib, c0:c0 + w],
                                             in0=sg_[:, :w], in1=pu[:, :w])

            def job_down(st, d_dram, accum):
                jn, hts, nib = st["jn"], st["hts"], st["nib"]
                for pass_ in range(2):
                    wds = []
                    for ib in range(nib):
                        wd = wdp.tile([P, 1024], bf16, tag=f"wd{ib}",
                                      name=f"wd_{jn}_{pass_}_{ib}")
                        nc.sync.dma_start(
                            out=wd[:],
                            in_=d_dram[ib * P:(ib + 1) * P,
                                       pass_ * 1024:(pass_ + 1) * 1024])
                        wds.append(wd)
                    for t in range(st["n"]):
                        yb = ybp.tile([P, 1024], bf16, tag="yb",
                                      name=f"yb_{jn}_{pass_}_{t}")
                        for hcl in range(2):
                            psy = psyp.tile([P, 512], f32, tag="py")
                            for ib in range(nib):
                                nc.tensor.matmul(
                                    out=psy[:],
                                    lhsT=hts[:, ib, t * P:(t + 1) * P],
                                    rhs=wds[ib][:, hcl * 512:(hcl + 1) * 512],
                                    start=(ib == 0), stop=(ib == nib - 1))
                            nc.scalar.copy(out=yb[:, hcl * 512:(hcl + 1) * 512],
                                           in_=psy[:])
                        if st["routed"]:
                            nc.gpsimd.indirect_dma_start(
                                out=py[:, :],
                                out_offset=bass.IndirectOffsetOnAxis(
                                    ap=st["tid"][t][:, 0:1], axis=0),
                                in_=yb[:, :], in_offset=None,
                                element_offset=pass_ * 1024,
                                bounds_check=breg_t, oob_is_err=False,
                                compute_op=AluOpType.add)
                        elif accum:
                            nc.gpsimd.dma_start(
                                out=zsh0[t * P:(t + 1) * P,
                                         pass_ * 1024:(pass_ + 1) * 1024],
                                in_=yb[:], accum_op=AluOpType.add)
                        else:
                            nc.sync.dma_start(
                                out=zsh0[t * P:(t + 1) * P,
                                         pass_ * 1024:(pass_ + 1) * 1024],
                                in_=yb[:])

            # shared chunk 0: start gate/up before the dispatch emission so
            # the PE has work while the AllGather + id-scatters run
            s0 = job_begin("s0", NTS, False, 0, NS0)
            job_gateup(s0, SGU[0], 0, 9)

            # ---------------- dispatch: slots + id scatter ----------------
            mrows = [sm.tile([8, TSH], f32, tag="scr", name=f"mrows{e}",
                             bufs=1) for e in range(2)]
            for e in range(2):
                nc.gpsimd.indirect_dma_start(
                    out=mrows[e][:, :],
                    out_offset=None,
                    in_=mskT_all[:, :],
                    in_offset=bass.IndirectOffsetOnAxis(
                        ap=erows_sb[e][:, 0:1], axis=0),
                )
            si_tiles = []
            for e in range(2):
                mask_cols = sm.tile([P, NT], f32, tag=f"mc{e}", name=f"mc{e}",
                                    bufs=1)
                for j in range(NTS):
                    tp3 = pst.tile([P, 512], f32, tag="tp")
                    nc.tensor.transpose(
                        out=tp3[:, :8],
                        in_=mrows[e][:, j * P:(j + 1) * P],
                        identity=identF[:8, :8])
                    nc.vector.tensor_copy(out=mask_cols[:, j * 8:(j + 1) * 8],
                                          in_=tp3[:, :8])
                excl_ps = pst.tile([P, 512], f32, tag="tp")
                nc.tensor.matmul(out=excl_ps[:, :NT], lhsT=triEX[:],
                                 rhs=mask_cols[:], start=True, stop=True)
                excl = sm.tile([P, NT], f32, tag=f"excl{e}", name=f"excl{e}",
                               bufs=1)
                nc.vector.tensor_copy(out=excl[:], in_=excl_ps[:, :NT])
                cnt_ps = pst.tile([P, 512], f32, tag="tp")
                nc.tensor.matmul(out=cnt_ps[:NT, :1], lhsT=mask_cols[:],
                                 rhs=ones_col[:], start=True, stop=True)
                cnt = sm.tile([NT, 1], f32, tag="cnt", name=f"cnt{e}")
                nc.vector.tensor_copy(out=cnt[:], in_=cnt_ps[:NT, :1])
                base_ps = pst.tile([P, 512], f32, tag="tp")
                nc.tensor.matmul(out=base_ps[:NT, :1], lhsT=triEX[:NT, :NT],
                                 rhs=cnt[:], start=True, stop=True)
                base_sb = sm.tile([NT, 1], f32, tag="cnt", name=f"base{e}")
                nc.vector.tensor_copy(out=base_sb[:], in_=base_ps[:NT, :1])
                nc.scalar.dma_start(out=baseb[e][:], in_=base_sb[:])
                base_bc = sm.tile([P, NT], f32, tag=f"bc{e}", name=f"bc{e}",
                                  bufs=1)
                nc.scalar.dma_start(out=base_bc[:],
                                    in_=bass.AP(baseb[e], 0, [[0, P], [1, NT]]))
                nc.vector.tensor_add(out=excl[:], in0=excl[:], in1=base_bc[:])
                nc.vector.tensor_scalar(out=excl[:], in0=excl[:],
                                        scalar1=float(-BIG), scalar2=None,
                                        op0=AluOpType.add)
                nc.vector.tensor_mul(out=excl[:], in0=excl[:], in1=mask_cols[:])
                nc.vector.tensor_scalar(out=excl[:], in0=excl[:],
                                        scalar1=float(BIG), scalar2=None,
                                        op0=AluOpType.add)
                si_ = sm.tile([P, NT], i32, tag=f"si{e}", name=f"si{e}", bufs=1)
                nc.vector.tensor_copy(out=si_[:], in_=excl[:])
                si_tiles.append(si_)
            for col in range(NT):
                for e in range(2):
                    nc.gpsimd.indirect_dma_start(
                        out=tid_d[e][:, :],
                        out_offset=bass.IndirectOffsetOnAxis(
                            ap=si_tiles[e][:, col:col + 1], axis=0),
                        in_=idc[:, col:col + 1], in_offset=None,
                        bounds_check=breg_e[e], oob_is_err=False)

            # rest of shared chunk 0
            job_gateup(s0, SGU[0], 9, NS0)
            job_down(s0, SD[0], accum=False)

            # routed experts
            e0 = job_begin("e0", CT0, True, 0, NIB)
            job_gateup(e0, JGU[0], 0, NIB)
            job_down(e0, JD[0], accum=False)
            e1 = job_begin("e1", CT1, True, 1, NIB)
            job_gateup(e1, JGU[1], 0, NIB)
            job_down(e1, JD[1], accum=False)

            # shared chunk 1 (overlaps the ReduceScatter below; emitted
            # first so its engine streams are ahead of the collective)
            s1 = job_begin("s1", NTS, False, 1, NS1)
            job_gateup(s1, SGU[1], 0, NS1)
            job_down(s1, SD[1], accum=True)

            # ---------------- ReduceScatter ----------------
            nc.gpsimd.collective_compute(
                "ReduceScatter", AluOpType.add,
                replica_groups=[list(range(NCORES))],
                ins=[bass.AP(py, 0, [[H, T], [1, H]])],
                outs=[bass.AP(rs_out, 0, [[H, TSH], [1, H]])],
            )

            # ---------------- final combine ----------------
            for g in range(NTS):
                rt = dtp.tile([P, H], bf16, tag="dt", name=f"rt{g}")
                nc.sync.dma_start(out=rt[:], in_=rs_out[g * P:(g + 1) * P, :])
                z0 = finp.tile([P, H], bf16, tag="zz", name=f"z0_{g}",
                               bufs=1)
                nc.sync.dma_start(out=z0[:], in_=zsh0[g * P:(g + 1) * P, :])
                for hh in range(4):
                    ob = finp.tile([P, 512], f32, tag="ob", name=f"ob{g}_{hh}")
                    nc.vector.tensor_add(
                        out=ob[:], in0=rt[:, hh * 512:(hh + 1) * 512],
                        in1=z0[:, hh * 512:(hh + 1) * 512])
                    nc.sync.dma_start(
                        out=out[g * P:(g + 1) * P, hh * 512:(hh + 1) * 512],
                        in_=ob[:])

    _split_multi_waits(nc)
    return nc


def _pack_gu(G, U):
    """[I', H] gate and up weights -> [P, nib*2*NHS*P] bf16 lhsT-tiled:
    per ib tile, first NHS*P cols = gate lhsT slices (hs-major), then up.
    lhsT slice hs of tile ib: [p, c] = W[ib*128+c, hs*128+p]."""
    nib = G.shape[0] // P
    out = np.empty((P, nib, 2, NHS, P), dtype=np_bf16)
    for Wm, j in ((G, 0), (U, 1)):
        W4 = Wm.reshape(nib, P, NHS, P)           # [ib, c, hs, p]
        out[:, :, j, :, :] = W4.transpose(3, 0, 2, 1)
    return np.ascontiguousarray(out.reshape(P, nib * 2 * NHS * P))


def kernel(x, rg_w, ru_w, extra_scale, extra_bias, Wg, Wu, Wd, Sg, Su, Sd):
    x = np.ascontiguousarray(np.asarray(x, dtype=np.float32))
    assert np.all(np.asarray(extra_scale) == 0.0), "kernel assumes extra_scale==0"
    assert np.all(np.asarray(extra_bias) == 0.0), "kernel assumes extra_bias==0"
    B, S, _ = x.shape
    xf = x.reshape(T, H)

    rg_w = np.asarray(rg_w, np.float32)
    ru_w = np.asarray(ru_w, np.float32)
    Wg = np.asarray(Wg, np.float32)
    Wu = np.asarray(Wu, np.float32)
    Wd = np.asarray(Wd, np.float32)
    Sg = np.asarray(Sg, np.float32)
    Su = np.asarray(Su, np.float32)
    Sd = np.asarray(Sd, np.float32)

    # host routing (fp64): exact top-4 counts -> static capacities.
    g = xf.astype(np.float64) @ rg_w.astype(np.float64).T
    u = xf.astype(np.float64) @ ru_w.astype(np.float64).T
    scores = np.abs(u * (g / (1.0 + np.exp(-g))))
    part = np.argpartition(-scores, K, axis=1)[:, :K]
    cnt = np.bincount(part.ravel(), minlength=E)
    ss = np.sort(scores, axis=1)[:, ::-1]
    gap = (ss[:, K - 1] - ss[:, K]) / np.maximum(ss[:, K - 1], 1e-30)
    assert gap.min() > 1e-5, f"top-4 margin too small: {gap.min()}"

    order = np.argsort(-cnt, kind="stable")
    CT0 = -(-int(cnt[order[0]]) // P)
    CT1 = -(-int(cnt[order[8]]) // P)

    key = (CT0, CT1)
    if _cached.get("key") != key:
        _cached.clear()
        _cached["key"] = key
        _cached["nc"] = build(CT0, CT1)
    nc = _cached["nc"]

    x_bf = xf.astype(np_bf16)
    rw32 = np.concatenate([rg_w, ru_w], axis=0)          # [32, H]
    rwp = np.ascontiguousarray(
        rw32.T.reshape(NHS, P, 32).transpose(1, 0, 2).reshape(P, NHS * 32)
    ).astype(np.float32)
    goff = (((np.arange(NT) % NTS) * TSH)
            + ((np.arange(NT) // NTS) * P)).astype(np.int32)

    SdT = np.ascontiguousarray(Sd.T)                     # [ISH, H]
    B0 = NS0 * P
    sgu_pack = [_pack_gu(Sg[:B0], Su[:B0]), _pack_gu(Sg[B0:], Su[B0:])]
    sd_pack = [np.ascontiguousarray(SdT[:B0]).astype(np_bf16),
               np.ascontiguousarray(SdT[B0:]).astype(np_bf16)]

    in_maps = []
    for c in range(NCORES):
        ea, eb = int(order[c]), int(order[15 - c])
        xsh = xf[c * TSH:(c + 1) * TSH]
        m = {
            "xbf": x_bf,
            "xtf": np.ascontiguousarray(xsh.T),
            "xtb": np.ascontiguousarray(xsh.T).astype(np_bf16),
            "rwp": rwp,
            "goff": goff,
            "erows": (np.array([cc * 16 + ea for cc in range(NCORES)]
                               + [cc * 16 + eb for cc in range(NCORES)],
                               dtype=np.int32).reshape(16, 1)),
            "JGU0": _pack_gu(Wg[ea], Wu[ea]),
            "JGU1": _pack_gu(Wg[eb], Wu[eb]),
            "JD0": np.ascontiguousarray(Wd[ea].T).astype(np_bf16),
            "JD1": np.ascontiguousarray(Wd[eb].T).astype(np_bf16),
            "SGU0": sgu_pack[0],
            "SGU1": sgu_pack[1],
            "SD0": sd_pack[0],
            "SD1": sd_pack[1],
        }
        in_maps.append(m)

    _cached["in_maps"] = in_maps
    res = run_bass_kernel_spmd(nc, in_maps, list(range(NCORES))).results
    yf = np.concatenate([res[c]["out"] for c in range(NCORES)], axis=0)
    return yf.reshape(B, S, H)
